# Initial kernel scaffold
#
"""Bahdanau-style attention with coverage on 8 Trainium2 NeuronCores.

Data-parallel over batch B=64: 8 batches per core, weights replicated.

Per batch b (all [m,l]-transposed "featT" layout, m on partitions):
  featT[m,l] = sum_n WhT[n,m]*hT[n,l]  (PE, f32r, 8 K-tiles)
             + Wc[m]*cov[l]            (PE, K=1 rank-1 augmentation)
  tanh via ACT with per-partition bias = bias[m] + (W_s @ s_t)[m]
  scores[l]  = sum_m v[m]*tanhfeatT[m,l]   (PE, v as stationary column)
  softmax over the [1,L] row (DVE/ACT), context = sum_l attn[l]*hT[n,l]
  via DVE tensor_tensor_reduce on the fp32 bits of hT.
"""

import numpy as np

import concourse.bass as bass  # noqa: F401  (registers engine classes)
import concourse.mybir as mybir
import concourse.tile as tile
from concourse import bacc
from concourse.bass_utils import run_bass_kernel_spmd

F32 = mybir.dt.float32
F32R = mybir.dt.float32r
AF = mybir.ActivationFunctionType
ALU = mybir.AluOpType
AX = mybir.AxisListType

B, L, N = 64, 1024, 1024
NCORES = 8
BSH = B // NCORES  # batches per core
NT = N // 128  # 128-row tiles along n / m
LHALF = 512  # moving-dim chunk (one PSUM bank of fp32)


def build_nc():
    nc = bacc.Bacc("TRN2", target_bir_lowering=False, debug=False, num_devices=NCORES)
    ht = nc.declare_dram_parameter("ht", [BSH, N, L], F32R, isOutput=False)
    cov = nc.declare_dram_parameter("cov", [BSH, L], F32R, isOutput=False)
    stT = nc.declare_dram_parameter("stT", [N, BSH], F32R, isOutput=False)
    whT = nc.declare_dram_parameter("whT", [N, N], F32R, isOutput=False)
    wsT = nc.declare_dram_parameter("wsT", [N, N], F32R, isOutput=False)
    wc = nc.declare_dram_parameter("wc", [1, N], F32R, isOutput=False)
    vv = nc.declare_dram_parameter("vv", [128, NT], F32R, isOutput=False)
    bia = nc.declare_dram_parameter("bia", [128, NT], F32, isOutput=False)
    attn_o = nc.declare_dram_parameter("attn", [BSH, L], F32, isOutput=True)
    ctx_o = nc.declare_dram_parameter("ctx", [BSH, N], F32, isOutput=True)
    covn_o = nc.declare_dram_parameter("covn", [BSH, L], F32, isOutput=True)

    with tile.TileContext(nc) as tc:
        with (
            tc.tile_pool(name="consts", bufs=1) as consts,
            tc.tile_pool(name="htp", bufs=2) as htp,
            tc.tile_pool(name="tfp", bufs=1) as tfp,
            tc.tile_pool(name="rows", bufs=3) as rows,
            tc.tile_pool(name="bcast", bufs=2) as bcastp,
            tc.tile_pool(name="scratch", bufs=2) as scratch,
            tc.tile_pool(name="psf", bufs=2, space="PSUM") as psf,
            tc.tile_pool(name="pssm", bufs=1, space="PSUM") as pssm,
            tc.tile_pool(name="pssc", bufs=2, space="PSUM") as pssc,
        ):
            whT_sb = consts.tile([128, NT, N], F32R)
            nc.sync.dma_start(
                out=whT_sb, in_=whT[:, :].rearrange("(t p) m -> p t m", p=128)
            )
            wsT_sb = consts.tile([128, NT, N], F32R)
            nc.sync.dma_start(
                out=wsT_sb, in_=wsT[:, :].rearrange("(t p) m -> p t m", p=128)
            )
            stT_sb = consts.tile([128, NT, BSH], F32R)
            nc.sync.dma_start(
                out=stT_sb, in_=stT[:, :].rearrange("(t p) b -> p t b", p=128)
            )
            wc_sb = consts.tile([1, N], F32R)
            nc.sync.dma_start(out=wc_sb, in_=wc[:, :])
            vv_sb = consts.tile([128, NT], F32R)
            nc.sync.dma_start(out=vv_sb, in_=vv[:, :])
            bia_sb = consts.tile([128, NT], F32)
            nc.sync.dma_start(out=bia_sb, in_=bia[:, :])

            # decoder-state projection: bcol[p, m_t, b] = bias[m] + (W_s @ s_t[b])[m]
            bcol_sb = consts.tile([128, NT, BSH], F32)
            for m_t in range(NT):
                psp = pssm.tile([128, BSH], F32, tag="psp")
                for n_t in range(NT):
                    nc.tensor.matmul(
                        psp[:, :],
                        wsT_sb[:, n_t, 128 * m_t : 128 * (m_t + 1)],
                        stT_sb[:, n_t, :],
                        start=(n_t == 0),
                        stop=(n_t == NT - 1),
                    )
                nc.scalar.add(bcol_sb[:, m_t, :], psp[:, :], bia_sb[:, m_t : m_t + 1])

            for b in range(BSH):
                ht_sb = htp.tile([128, NT, L], F32R, tag="ht")
                nc.sync.dma_start(
                    out=ht_sb, in_=ht[b].rearrange("(t p) l -> p t l", p=128)
                )
                covr = rows.tile([1, L], F32R, tag="covr")
                nc.sync.dma_start(out=covr, in_=cov[b : b + 1, :])

                tf_sb = tfp.tile([128, NT, L], F32R, tag="tf")
                for m_t in range(NT):
                    pf = psf.tile([128, L], F32, tag="pf")
                    for lh in range(2):
                        sl = slice(LHALF * lh, LHALF * (lh + 1))
                        for n_t in range(NT):
                            nc.tensor.matmul(
                                pf[:, sl],
                                whT_sb[:, n_t, 128 * m_t : 128 * (m_t + 1)],
                                ht_sb[:, n_t, sl],
                                start=(n_t == 0),
                                stop=False,
                            )
                        nc.tensor.matmul(
                            pf[:, sl],
                            wc_sb[:, 128 * m_t : 128 * (m_t + 1)],
                            covr[:, sl],
                            start=False,
                            stop=True,
                        )
                    nc.scalar.activation(
                        tf_sb[:, m_t, :],
                        pf[:, :],
                        AF.Tanh,
                        bias=bcol_sb[:, m_t, b : b + 1],
                        scale=1.0,
                    )

                sc_sb = rows.tile([1, L], F32, tag="sc")
                for lh in range(2):
                    sl = slice(LHALF * lh, LHALF * (lh + 1))
                    psc = pssc.tile([1, LHALF], F32, tag="psc")
                    for m_t in range(NT):
                        nc.tensor.matmul(
                            psc[:, :],
                            vv_sb[:, m_t : m_t + 1],
                            tf_sb[:, m_t, sl],
                            start=(m_t == 0),
                            stop=(m_t == NT - 1),
                        )
                    nc.vector.tensor_copy(sc_sb[:, sl], psc[:, :])

                # softmax over the [1, L] row
                nmx = rows.tile([1, 1], F32, tag="nmx")
                nc.vector.tensor_reduce(
                    nmx, sc_sb, axis=AX.X, op=ALU.max, negate=True
                )
                esc = rows.tile([1, L], F32, tag="esc")
                nc.scalar.activation(esc, sc_sb, AF.Exp, bias=nmx[:, :], scale=1.0)
                ssum = rows.tile([1, 1], F32, tag="ssum")
                nc.vector.reduce_sum(ssum, esc, axis=AX.X)
                rsum = rows.tile([1, 1], F32, tag="rsum")
                nc.vector.reciprocal(rsum, ssum)
                attn_r = rows.tile([1, L], F32, tag="attn")
                nc.vector.tensor_scalar_mul(attn_r, esc, rsum[:, :])
                nc.sync.dma_start(out=attn_o[b : b + 1, :], in_=attn_r)

                covn_r = rows.tile([1, L], F32, tag="covn")
                nc.vector.tensor_add(covn_r, covr.bitcast(F32), attn_r)
                nc.sync.dma_start(out=covn_o[b : b + 1, :], in_=covn_r)

                # context[n] = sum_l attn[l] * h[l, n], per n-strip on DVE
                abc = bcastp.tile([128, L], F32, tag="abc")
                nc.gpsimd.partition_broadcast(abc, attn_r)
                ctx_cols = bcastp.tile([128, NT], F32, tag="ctxc")
                scr = scratch.tile([128, L], F32, tag="scr")
                for s in range(NT):
                    nc.vector.tensor_tensor_reduce(
                        out=scr[:, :],
                        in0=ht_sb[:, s, :].bitcast(F32),
                        in1=abc[:, :],
                        scale=1.0,
                        scalar=0.0,
                        op0=ALU.mult,
                        op1=ALU.add,
                        accum_out=ctx_cols[:, s : s + 1],
                    )
                nc.sync.dma_start(
                    out=ctx_o[b].rearrange("(t p) -> p t", p=128), in_=ctx_cols
                )

    nc.compile()
    return nc


_NC_CACHE = None


def _get_nc():
    global _NC_CACHE
    if _NC_CACHE is None:
        _NC_CACHE = build_nc()
    return _NC_CACHE


def _prep_in_maps(h, s_t, coverage, W_h, W_s, W_c, v, bias):
    hT = np.ascontiguousarray(h.transpose(0, 2, 1), dtype=np.float32)
    stT = np.ascontiguousarray(s_t.T, dtype=np.float32)  # [N, B]
    whT = np.ascontiguousarray(W_h.T, dtype=np.float32)
    wsT = np.ascontiguousarray(W_s.T, dtype=np.float32)
    wc = np.ascontiguousarray(W_c[:, 0].reshape(1, N), dtype=np.float32)
    vv = np.ascontiguousarray(v.reshape(NT, 128).T, dtype=np.float32)  # [128, NT]
    bia = np.ascontiguousarray(bias.reshape(NT, 128).T, dtype=np.float32)
    in_maps = []
    for c in range(NCORES):
        sl = slice(c * BSH, (c + 1) * BSH)
        in_maps.append(
            {
                "ht": np.ascontiguousarray(hT[sl]),
                "cov": np.ascontiguousarray(coverage[sl], dtype=np.float32),
                "stT": np.ascontiguousarray(stT[:, sl]),
                "whT": whT,
                "wsT": wsT,
                "wc": wc,
                "vv": vv,
                "bia": bia,
            }
        )
    return in_maps


def run(trace=False, **inputs):
    nc = _get_nc()
    in_maps = _prep_in_maps(**{k: np.asarray(v) for k, v in inputs.items()})
    res = run_bass_kernel_spmd(
        nc, in_maps, core_ids=list(range(NCORES)), trace=trace
    )
    attn = np.concatenate([r["attn"] for r in res.results], axis=0)
    ctx = np.concatenate([r["ctx"] for r in res.results], axis=0)
    covn = np.concatenate([r["covn"] for r in res.results], axis=0)
    return (attn, ctx, covn), res


def kernel(**inputs):
    outs, _ = run(trace=False, **inputs)
    return outs


# revision 8
# speedup vs baseline: 1.2667x; 1.2667x over previous
"""Bahdanau-style attention with coverage on 8 Trainium2 NeuronCores.

Data-parallel over batch B=64: 8 batches per core, weights replicated.

Per batch b (all [m,l]-transposed "featT" layout, m on partitions):
  featT[m,l] = sum_n WhT[n,m]*hT[n,l]  (PE, f32r, 8 K-tiles)
             + Wc[m]*cov[l]            (PE, K=1 rank-1 augmentation)
  tanh via ACT with per-partition bias = bias[m] + (W_s @ s_t)[m]
  scores[l]  = sum_m v[m]*tanhfeatT[m,l]   (PE, v as stationary column)
  softmax over the [1,L] row (DVE/ACT), context = sum_l attn[l]*hT[n,l]
  via DVE tensor_tensor_reduce on the fp32 bits of hT.
"""

import ml_dtypes
import numpy as np

import concourse.bass as bass  # noqa: F401  (registers engine classes)
import concourse.mybir as mybir
import concourse.tile as tile
from concourse import bacc
from concourse.bass_utils import run_bass_kernel_spmd

F32 = mybir.dt.float32
F32R = mybir.dt.float32r
BF16 = mybir.dt.bfloat16
AF = mybir.ActivationFunctionType
ALU = mybir.AluOpType
AX = mybir.AxisListType

B, L, N = 64, 1024, 1024
NCORES = 8
BSH = B // NCORES  # batches per core
NT = N // 128  # 128-row tiles along n / m
LHALF = 512  # moving-dim chunk (one PSUM bank of fp32)


def build_nc(reps: int = 1):
    nc = bacc.Bacc("TRN2", target_bir_lowering=False, debug=False, num_devices=NCORES)
    ht = nc.declare_dram_parameter("ht", [BSH, N, L], F32, isOutput=False)
    cov = nc.declare_dram_parameter("cov", [BSH, L], F32, isOutput=False)
    stT = nc.declare_dram_parameter("stT", [N, BSH], F32R, isOutput=False)
    whT = nc.declare_dram_parameter("whT", [N, N], F32R, isOutput=False)
    wsT = nc.declare_dram_parameter("wsT", [N, N], F32R, isOutput=False)
    wc = nc.declare_dram_parameter("wc", [1, N], F32R, isOutput=False)
    vv = nc.declare_dram_parameter("vv", [128, NT], BF16, isOutput=False)
    bia = nc.declare_dram_parameter("bia", [128, NT], F32, isOutput=False)
    attn_o = nc.declare_dram_parameter("attn", [BSH, L], F32, isOutput=True)
    ctx_o = nc.declare_dram_parameter("ctx", [BSH, N], F32, isOutput=True)
    covn_o = nc.declare_dram_parameter("covn", [BSH, L], F32, isOutput=True)

    with tile.TileContext(nc) as tc:
        with tc.tile_pool(name="consts", bufs=1) as consts:
            whT_sb = consts.tile([128, NT, N], F32R)
            nc.sync.dma_start(
                out=whT_sb, in_=whT[:, :].rearrange("(t p) m -> p t m", p=128)
            )
            wc_sb = consts.tile([1, N], F32R)
            nc.sync.dma_start(out=wc_sb, in_=wc[:, :])
            vv_sb = consts.tile([128, NT], BF16)
            nc.sync.dma_start(out=vv_sb, in_=vv[:, :])
            bia_sb = consts.tile([128, NT], F32)
            nc.sync.dma_start(out=bia_sb, in_=bia[:, :])

            # decoder-state projection: bcol[p, m_t, b] = bias[m] + (W_s @ s_t[b])[m]
            bcol_sb = consts.tile([128, NT, BSH], F32)
            with (
                tc.tile_pool(name="sproj", bufs=1) as sprojp,
                tc.tile_pool(name="pssm", bufs=2, space="PSUM") as pssm,
            ):
                wsT_sb = sprojp.tile([128, NT, N], F32R)
                nc.sync.dma_start(
                    out=wsT_sb, in_=wsT[:, :].rearrange("(t p) m -> p t m", p=128)
                )
                stT_sb = sprojp.tile([128, NT, BSH], F32R)
                nc.sync.dma_start(
                    out=stT_sb, in_=stT[:, :].rearrange("(t p) b -> p t b", p=128)
                )
                for m_t in range(NT):
                    psp = pssm.tile([128, BSH], F32, tag="psp")
                    for n_t in range(NT):
                        nc.tensor.matmul(
                            psp[:, :],
                            wsT_sb[:, n_t, 128 * m_t : 128 * (m_t + 1)],
                            stT_sb[:, n_t, :],
                            start=(n_t == 0),
                            stop=(n_t == NT - 1),
                        )
                    nc.scalar.add(
                        bcol_sb[:, m_t, :], psp[:, :], bia_sb[:, m_t : m_t + 1]
                    )

            main_pools = (
                tc.tile_pool(name="htp", bufs=2),
                tc.tile_pool(name="htr", bufs=1),
                tc.tile_pool(name="tfp", bufs=1),
                tc.tile_pool(name="rows", bufs=2),
                tc.tile_pool(name="bcast", bufs=2),
                tc.tile_pool(name="scratch", bufs=2),
                tc.tile_pool(name="dramp", bufs=2, space="DRAM"),
                tc.tile_pool(name="psf", bufs=2, space="PSUM"),
                tc.tile_pool(name="pssc", bufs=2, space="PSUM"),
            )
            import contextlib

            stack = contextlib.ExitStack()
            htp, htrp, tfp, rows, bcastp, scratch, dramp, psf, pssc = (
                stack.enter_context(p) for p in main_pools
            )
            for b in [bb for _ in range(reps) for bb in range(BSH)]:
                ht_sb = htp.tile([128, NT, L], F32, tag="ht")
                nc.sync.dma_start(
                    out=ht_sb, in_=ht[b].rearrange("(t p) l -> p t l", p=128)
                )
                ht_r8 = htrp.tile([128, NT, L], F32R, tag="htr")
                for s_ in range(NT):
                    nc.gpsimd.tensor_copy(ht_r8[:, s_, :], ht_sb[:, s_, :])
                covr = rows.tile([1, L], F32, tag="covr")
                nc.sync.dma_start(out=covr, in_=cov[b : b + 1, :])
                covr_r = rows.tile([1, L], F32R, tag="covr_r")
                nc.gpsimd.tensor_copy(covr_r, covr)

                tf_sb = tfp.tile([128, NT, L], BF16, tag="tf")
                for m_t in range(NT):
                    pf = psf.tile([128, L], F32, tag="pf")
                    for lh in range(2):
                        sl = slice(LHALF * lh, LHALF * (lh + 1))
                        for n_t in range(NT):
                            nc.tensor.matmul(
                                pf[:, sl],
                                whT_sb[:, n_t, 128 * m_t : 128 * (m_t + 1)],
                                ht_r8[:, n_t, sl],
                                start=(n_t == 0),
                                stop=False,
                            )
                        nc.tensor.matmul(
                            pf[:, sl],
                            wc_sb[:, 128 * m_t : 128 * (m_t + 1)],
                            covr_r[:, sl],
                            start=False,
                            stop=True,
                        )
                    nc.scalar.activation(
                        tf_sb[:, m_t, :],
                        pf[:, :],
                        AF.Tanh,
                        bias=bcol_sb[:, m_t, b : b + 1],
                        scale=1.0,
                    )

                sc_sb = rows.tile([1, L], F32, tag="sc")
                for lh in range(2):
                    sl = slice(LHALF * lh, LHALF * (lh + 1))
                    psc = pssc.tile([1, LHALF], F32, tag="psc")
                    for m_t in range(NT):
                        nc.tensor.matmul(
                            psc[:, :],
                            vv_sb[:, m_t : m_t + 1],
                            tf_sb[:, m_t, sl],
                            start=(m_t == 0),
                            stop=(m_t == NT - 1),
                        )
                    nc.vector.tensor_copy(sc_sb[:, sl], psc[:, :])

                # softmax over the [1, L] row; stats live in spare ctx_cols cols
                ctx_cols = bcastp.tile([128, NT + 4], F32, tag="ctxc")
                nmx = ctx_cols[0:1, NT : NT + 1]
                nc.vector.tensor_reduce(
                    nmx, sc_sb, axis=AX.X, op=ALU.max, negate=True
                )
                attn_r = rows.tile([1, L], F32, tag="esc")
                nc.scalar.activation(attn_r, sc_sb, AF.Exp, bias=nmx, scale=1.0)
                ssum = ctx_cols[0:1, NT + 1 : NT + 2]
                nc.vector.reduce_sum(ssum, attn_r, axis=AX.X)
                rsum = ctx_cols[0:1, NT + 2 : NT + 3]
                nc.vector.reciprocal(rsum, ssum)
                nc.vector.tensor_scalar_mul(attn_r, attn_r, rsum)
                nc.sync.dma_start(out=attn_o[b : b + 1, :], in_=attn_r)

                covn_r = sc_sb  # scores row no longer needed; reuse as coverage_new
                nc.vector.tensor_add(covn_r, covr, attn_r)
                nc.sync.dma_start(out=covn_o[b : b + 1, :], in_=covn_r)

                # context[n] = sum_l attn[l] * h[l, n], per n-strip on DVE.
                # Broadcast attn across partitions via a DRAM bounce (SBUF
                # sources cannot have partition-step-0 APs).
                abt = dramp.tile([1, L], F32, tag="abt")
                nc.sync.dma_start(out=abt, in_=attn_r)
                abc = bcastp.tile([128, L], F32, tag="abc")
                nc.sync.dma_start(
                    out=abc, in_=abt[:, :].partition_broadcast(128).squeeze(1)
                )
                scr = scratch.tile([128, L], F32, tag="scr")
                for s in range(NT):
                    nc.vector.tensor_mul(scr[:, :], ht_sb[:, s, :], abc[:, :])
                    nc.vector.reduce_sum(
                        ctx_cols[:, s : s + 1], scr[:, :], axis=AX.X
                    )
                nc.sync.dma_start(
                    out=ctx_o[b].rearrange("(t p) -> p t", p=128),
                    in_=ctx_cols[:, 0:NT],
                )
            stack.close()

    nc.compile()
    return nc


_NC_CACHE = {}


def _get_nc(reps: int = 1):
    if reps not in _NC_CACHE:
        _NC_CACHE[reps] = build_nc(reps)
    return _NC_CACHE[reps]


def _prep_in_maps(h, s_t, coverage, W_h, W_s, W_c, v, bias):
    hT = np.ascontiguousarray(h.transpose(0, 2, 1), dtype=np.float32)
    stT = np.ascontiguousarray(s_t.T, dtype=np.float32)  # [N, B]
    whT = np.ascontiguousarray(W_h.T, dtype=np.float32)
    wsT = np.ascontiguousarray(W_s.T, dtype=np.float32)
    wc = np.ascontiguousarray(W_c[:, 0].reshape(1, N), dtype=np.float32)
    vv = np.ascontiguousarray(v.reshape(NT, 128).T).astype(ml_dtypes.bfloat16)
    bia = np.ascontiguousarray(bias.reshape(NT, 128).T, dtype=np.float32)
    in_maps = []
    for c in range(NCORES):
        sl = slice(c * BSH, (c + 1) * BSH)
        in_maps.append(
            {
                "ht": np.ascontiguousarray(hT[sl]),
                "cov": np.ascontiguousarray(coverage[sl], dtype=np.float32),
                "stT": np.ascontiguousarray(stT[:, sl]),
                "whT": whT,
                "wsT": wsT,
                "wc": wc,
                "vv": vv,
                "bia": bia,
            }
        )
    return in_maps


def run(trace=False, **inputs):
    nc = _get_nc()
    in_maps = _prep_in_maps(**{k: np.asarray(v) for k, v in inputs.items()})
    res = run_bass_kernel_spmd(
        nc, in_maps, core_ids=list(range(NCORES)), trace=trace
    )
    attn = np.concatenate([r["attn"] for r in res.results], axis=0)
    ctx = np.concatenate([r["ctx"] for r in res.results], axis=0)
    covn = np.concatenate([r["covn"] for r in res.results], axis=0)
    return (attn, ctx, covn), res


def kernel(**inputs):
    outs, _ = run(trace=False, **inputs)
    return outs
